# revision 27
# baseline (speedup 1.0000x reference)
"""Distributed causal multi-head attention for Trainium2 (8 NeuronCores).

Problem: B=2, T=2048, D=1024, 16 heads, head_dim=64, fp32 reference.
  q/k/v = x @ W{q,k,v}.T ; per-head causal softmax(q k^T/8) v ; out @ Wo.T

Sharding: tensor-parallel over heads -- core c owns heads {2c, 2c+1}.
Per core (bf16 storage, fp32 PSUM accumulation):
  - x^T fully resident in SBUF; chunk 0 + weights DMA first, later chunks
    are gated on compute progress (1-elem ACT adds create WAR deps) so the
    DMA rings don't bandwidth-share the critical first chunk.
  - Q^T, K^T, V^T = W_shard @ x^T (weight-stationary, N=512 streams); V^T
    is then PE-transposed (identity matmul) into token-major VA with a
    fused ones-column so PV also produces the softmax denominator.
  - scores as S^T [k, q]: K^T block stationary, Q^T moving, 2 heads
    row-packed at partition offsets 0/64 (concurrent); exp on ACT with
    causal block skipping; the causal edge is masked POST-exp by a 0/1
    lower-tri multiply on DVE (never on GpSimd: collective triggers spin
    there and would hold the mask hostage). PV accumulates [sumexp; O^T].
  - projection/o-proj matmuls are generator-based filler tasks interleaved
    ~2 MMs per kb step, so the in-order PE eats independent work while ACT
    computes exp and the HAM clock gate stays warm.
  - normalize O^T by 1/sumexp: PSUM->SBUF copy, DVE fast reciprocal, K=1
    ones-matmul partition broadcast (f32r), one fused DVE multiply; each
    unit's normalization is deferred into the next unit's pipeline.
  - 4-round AllToAll (one per q-chunk) re-shards feature-split ->
    token-split; a dummy collective at t~0 absorbs the one-time CC
    barrier/ring setup; ALL o-proj rounds + a few warmup matmuls are
    deferred behind the last AllToAll trigger so the PE stays busy (and
    the clock warm) while the tail exchange is in flight. Each core
    outputs a [512, 1024] slice; host reassembles.
"""

import functools
import numpy as np
import ml_dtypes

import concourse.bass as bass
from concourse.bass import ds
import concourse.mybir as mybir
import concourse.tile as tile
from concourse import bacc
from concourse import bass_utils

F32 = mybir.dt.float32
F32R = mybir.dt.float32r
BF16 = mybir.dt.bfloat16

P = 128
NCORES = 8
B, T, DIM = 2, 2048, 1024
NH, HD = 16, 64
TOK = B * T               # 4096 flattened tokens
NKT = DIM // P            # 8 contraction tiles
NTC = TOK // 512          # 8 token chunks of 512
CPB = 4                   # q-chunks of 512 per batch
KPB = 16                  # 128-wide k-blocks per batch
SLICE = TOK // NCORES     # 512 output tokens per core


def build_kernel(debug=False, no_cc=False):
    nc = bacc.Bacc("TRN2", num_devices=NCORES)

    xT = nc.declare_dram_parameter("xT", [NTC, P, NKT, 512], BF16, isOutput=False)
    wq = nc.declare_dram_parameter("wq", [P, NKT, P], BF16, isOutput=False)
    wk = nc.declare_dram_parameter("wk", [P, NKT, P], BF16, isOutput=False)
    wv = nc.declare_dram_parameter("wv", [P, NKT, P], BF16, isOutput=False)
    wo = nc.declare_dram_parameter("wo", [P, NKT, DIM], BF16, isOutput=False)
    tri = nc.declare_dram_parameter("tri", [P, P], BF16, isOutput=False)
    idn = nc.declare_dram_parameter("idn", [P, P], BF16, isOutput=False)
    out = nc.declare_dram_parameter("out", [SLICE, DIM], F32, isOutput=True)

    with tile.TileContext(nc) as tc:
        # ---- resident SBUF ----
        res = tc.alloc_tile_pool(name="res", bufs=1)
        XT = res.tile([P, NTC, NKT, 512], BF16, name="XT")   # [in-feat, tok]
        QKT = res.tile([P, 2, NTC, 512], BF16, name="QKT")   # [feat, q/k, tok]
        VA = res.tile([P, TOK // P, 2, HD + 1], BF16, name="VA")
        WQ = res.tile([P, NKT, P], BF16, name="WQ")
        WK = res.tile([P, NKT, P], BF16, name="WK")
        WV = res.tile([P, NKT, P], BF16, name="WV")
        WO = res.tile([P, NKT, DIM], BF16, name="WO")
        TRI = res.tile([P, P], BF16, name="TRI")
        IDN = res.tile([P, P], BF16, name="IDN")
        ONESF = res.tile([P, HD + 1], F32, name="ONESF")
        ONES = res.tile([P, HD + 1], F32R, name="ONES")
        CCS = res.tile([P, 8], BF16, name="CCS")
        GS = res.tile([P, 16], F32, name="GS")

        nc.vector.memset(ONESF[:], 1.0)
        nc.vector.tensor_copy(out=ONES[:], in_=ONESF[:])
        nc.vector.memset(VA[:, :, :, 0:1], 1.0)
        nc.gpsimd.memset(CCS[:], 1.0)

        # per-round A2A bounce buffers (bf16) + dummy-collective scratch
        a2a_in = []
        a2a_out = []
        frees = []
        for r in range(CPB):
            ai, f1 = tc.tile([B, CPB, P, P], BF16, space="DRAM",
                             name=f"a2a_in{r}")
            ao, f2 = tc.tile([NKT, P, P], BF16, space="DRAM",
                             name=f"a2a_out{r}", addr_space="Shared")
            a2a_in.append(ai)
            a2a_out.append(ao)
            frees += [f1, f2]
        cc_in, f1 = tc.tile([P, 8], BF16, space="DRAM", name="cc_in")
        cc_out, f2 = tc.tile([P, 8], BF16, space="DRAM", name="cc_out",
                             addr_space="Shared")
        frees += [f1, f2]

        with (
            tc.tile_pool(name="sc_ps", bufs=2, space="PSUM") as sc_ps,
            tc.tile_pool(name="ot_ps", bufs=1, space="PSUM") as ot_ps,
            tc.tile_pool(name="m_ps", bufs=2, space="PSUM") as m_ps,
            tc.tile_pool(name="ptp", bufs=3) as ptp,
            tc.tile_pool(name="nrm", bufs=2) as nrm,
            tc.tile_pool(name="att", bufs=2) as atp,
        ):
            # ---- upfront DMAs ----
            # dummy-collective input first: its barrier must start at t~0
            nc.sync.dma_start(cc_in[:, :], CCS[:, :])
            if not no_cc:
                nc.gpsimd.collective_compute(
                    "AllToAll", mybir.AluOpType.bypass,
                    replica_groups=[list(range(NCORES))],
                    ins=[cc_in[:, :].opt()],
                    outs=[cc_out[:, :].opt()],
                )
            # chunk 0 + weights immediately (attention starts on these);
            # later chunks are gated on compute progress below so the DMA
            # rings don't bandwidth-share chunk 0 into a 15us ramp.
            nc.sync.dma_start(XT[:, 0, 0:4, :], xT[0, :, 0:4, :])
            nc.sync.dma_start(WQ[:], wq[:, :, :])
            nc.sync.dma_start(XT[:, 0, 4:8, :], xT[0, :, 4:8, :])
            nc.sync.dma_start(WK[:], wk[:, :, :])
            nc.sync.dma_start(WV[:], wv[:, :, :])
            nc.sync.dma_start(TRI[:], tri[:, :])
            nc.sync.dma_start(IDN[:], idn[:, :])
            # chunk to load when each chunk's Q-cast lands (None = WO)
            next_load = {0: [4], 4: [1], 1: [5], 5: [2, 6], 2: [3, 7],
                         6: [None]}
            gs_n = [0]

            def emit_gated_load(qch):
                for ld in next_load.pop(qch, ()):
                    # WAR gate: 1-elem ACT add reads the load target + the
                    # just-produced Q row; the bulk DMA then waits on it.
                    gs_i = gs_n[0]
                    gs_n[0] += 1
                    if ld is None:
                        nc.scalar.add(GS[0:1, gs_i:gs_i + 1],
                                      WO[0:1, 0, 0:1],
                                      QKT[0:1, 0, qch, 0:1])
                        nc.sync.dma_start(WO[:], wo[:, :, :])
                    else:
                        nc.scalar.add(GS[0:1, gs_i:gs_i + 1],
                                      XT[0:1, ld, 0, 0:1],
                                      QKT[0:1, 0, qch, 0:1])
                        nc.sync.dma_start(XT[:, ld, :, :], xT[ld, :, :, :])

            # PE warmup on garbage data while the first x chunk is in
            # flight: no input deps, so these issue immediately and carry
            # the HAM activity window past its 3.4us threshold before real
            # matmuls arrive. WO is not DMA'd until much later (gated), so
            # reading it creates no dependency on any in-flight load.
            for g in range(4):
                wm = m_ps.tile([P, 512], F32, tag="m", name="wm")
                for kt in range(5):
                    nc.tensor.matmul(wm[:], lhsT=WO[:, 0, kt * P:(kt + 1) * P],
                                     rhs=WO[:, 1, 0:512],
                                     start=(kt == 0), stop=(kt == 4))

            # ---- filler task queue: generators yielding every ~2 MMs ----
            vts_state = {}

            def g_proj(ch, which):
                # which: 0=Q, 1=K, 2=V^T
                W = (WQ, WK, WV)[which]
                m = m_ps.tile([P, 512], F32, tag="m", name="mp")
                for kt in range(NKT):
                    nc.tensor.matmul(m[:], lhsT=W[:, kt, :],
                                     rhs=XT[:, ch, kt, :],
                                     start=(kt == 0), stop=(kt == NKT - 1))
                    if kt % 2 == 1 and kt < NKT - 1:
                        yield
                if which == 2:
                    vts = atp.tile([P, 512], BF16, tag="vt", name="vts")
                    nc.vector.tensor_copy(out=vts[:], in_=m[:])
                    vts_state[ch] = vts
                else:
                    nc.vector.tensor_copy(out=QKT[:, which, ch, :], in_=m[:])
                    if which == 0:
                        emit_gated_load(ch)

            def g_vtr(ch):
                # PE-transpose V^T -> V [tok, feat] (ones column preset)
                vts = vts_state.pop(ch)
                m = m_ps.tile([P, CPB, 2, HD], BF16, tag="m", name="mtr")
                for i in range(CPB):
                    nc.tensor.transpose(m[:, i, :, :],
                                        vts[:, i * P:(i + 1) * P],
                                        IDN[:, :])
                    if i == 1:
                        yield
                tt0 = 4 * ch
                nc.vector.tensor_copy(
                    out=VA[:, tt0:tt0 + 4, :, 1:HD + 1], in_=m[:])

            tasks = []
            for ch in (0, 4, 1, 5, 2, 6, 3, 7):
                tasks.append((ch, lambda ch=ch: g_proj(ch, 0)))
                tasks.append((ch, lambda ch=ch: g_proj(ch, 1)))
                tasks.append((ch, lambda ch=ch: g_proj(ch, 2)))
                tasks.append((ch, lambda ch=ch: g_vtr(ch)))
            tpos = [0]
            cur = [None, None]   # (generator, ch)
            done_ch = set()
            done_cnt = {}

            def _advance():
                # one micro-step (~2 MMs); returns False when queue empty
                if cur[0] is None:
                    if tpos[0] >= len(tasks):
                        return False
                    ch, mk = tasks[tpos[0]]
                    tpos[0] += 1
                    cur[0], cur[1] = mk(), ch
                try:
                    next(cur[0])
                except StopIteration:
                    ch = cur[1]
                    cur[0] = None
                    done_cnt[ch] = done_cnt.get(ch, 0) + 1
                    if tpos[0] >= len(tasks) or tasks[tpos[0]][0] != ch:
                        done_ch.add(ch)
                return True

            def drain_micro(n):
                for _ in range(n):
                    if not _advance():
                        return

            def drain_task_boundary():
                while cur[0] is not None:
                    _advance()

            def ensure(chunks):
                while not all(c in done_ch for c in chunks):
                    if not _advance():
                        return

            def ensure_qk(chunks):
                # enough to start scores: Q and K of each chunk (first two
                # tasks of its group); V^T/transpose drain in-loop before
                # the first PV consumes them.
                def ok(ch):
                    return ch in done_ch or done_cnt.get(ch, 0) >= 2
                while not all(ok(ch) for ch in chunks):
                    if not _advance():
                        return

            def emit_oproj(r):
                att2 = atp.tile([P, NKT, P], BF16, tag="att", name="att2")
                nc.sync.dma_start(att2[:, :, :],
                                  a2a_out[r][:, :, :].transpose([1, 0, 2]))
                for oh in range(2):
                    m = m_ps.tile([P, 512], F32, tag="m", name="mo")
                    for kt in range(NKT):
                        nc.tensor.matmul(
                            m[:], lhsT=att2[:, kt, :],
                            rhs=WO[:, kt, oh * 512:(oh + 1) * 512],
                            start=(kt == 0), stop=(kt == NKT - 1))
                    osb = atp.tile([P, 512], F32, tag="osb", name="osb")
                    nc.vector.tensor_copy(out=osb[:], in_=m[:])
                    nc.sync.dma_start(
                        out[r * P:(r + 1) * P, oh * 512:(oh + 1) * 512],
                        osb[:])

            pending = []

            def flush_pending():
                while pending:
                    pending.pop(0)()

            def emit_attn(c, b):
                qch = b * CPB + c
                oA = ot_ps.tile([HD + 1, CPB, P], F32, tag="oA", name="oA")
                oB = ot_ps.tile([HD + 1, CPB, P], F32, tag="oB", name="oB")
                nkb = 4 * (c + 1)

                def emit_scores(kb):
                    kch = b * CPB + kb // 4
                    kcol = (kb % 4) * P
                    off = max(0, kb - 4 * c) * P
                    s_ps = sc_ps.tile([P, 2, 512], F32, tag="s", name="s_ps")
                    nc.tensor.matmul(
                        s_ps[:, 0, off:512],
                        lhsT=QKT[0:HD, 1, kch, kcol:kcol + P],
                        rhs=QKT[0:HD, 0, qch, off:512], start=True, stop=True)
                    nc.tensor.matmul(
                        s_ps[:, 1, off:512],
                        lhsT=QKT[HD:P, 1, kch, kcol:kcol + P],
                        rhs=QKT[HD:P, 0, qch, off:512], start=True, stop=True,
                        tile_position=(HD, 0))
                    return s_ps

                def emit_exp(kb, s_ps):
                    d = kb - 4 * c
                    off = max(0, d) * P
                    pt = ptp.tile([P, 2, 512], BF16, tag="p", name="pt")
                    nc.scalar.activation(
                        pt[:, :, off:512], s_ps[:, :, off:512],
                        mybir.ActivationFunctionType.Exp, scale=0.125)
                    if d >= 0:
                        # causal mask: zero upper triangle post-exp (bf16 DVE
                        # op; NOT on gpsimd -- collective triggers spin there
                        # and would hold the mask hostage)
                        nc.vector.tensor_mul(
                            out=pt[:, :, off:off + P],
                            in0=pt[:, :, off:off + P],
                            in1=TRI[:, None, :].to_broadcast([P, 2, P]))
                    return pt

                s_tiles = {0: emit_scores(0)}
                if nkb > 1:
                    s_tiles[1] = emit_scores(1)
                pt_tiles = {0: emit_exp(0, s_tiles.pop(0))}
                flush_pending()
                for kb in range(nkb):
                    if kb + 1 < nkb:
                        pt_tiles[kb + 1] = emit_exp(kb + 1, s_tiles.pop(kb + 1))
                    # filler BEFORE the dependent PV: the in-order PE eats
                    # these while ACT computes exp(kb)
                    drain_micro(2 if kb < nkb // 2 else 1)
                    if kb + 2 < nkb:
                        s_tiles[kb + 2] = emit_scores(kb + 2)
                    off = max(0, kb - 4 * c) * P
                    ob = off // P
                    pt = pt_tiles.pop(kb)
                    ktile = b * KPB + kb
                    nc.tensor.matmul(oA[:, ob:CPB, :],
                                     lhsT=VA[:, ktile, 0, :],
                                     rhs=pt[:, 0, off:512],
                                     start=(kb == 0), stop=(kb == nkb - 1))
                    nc.tensor.matmul(oB[:, ob:CPB, :],
                                     lhsT=VA[:, ktile, 1, :],
                                     rhs=pt[:, 1, off:512],
                                     start=(kb == 0), stop=(kb == nkb - 1))

                def finisher():
                    # normalize + scatter into round-c bounce buffer.
                    # phase 1: reciprocal straight from PSUM row 0, with the
                    # full-tile copy (only needed by the mul) in parallel --
                    # keeps the recip chain off the copy's latency.
                    # single rb alloc: a filler-task PSUM tile may be live,
                    # so only one extra m_ps slot may be taken here.
                    ocs, rrrs = [], []
                    for h, o_ps in ((0, oA), (1, oB)):
                        rr = nrm.tile([1, CPB, P], F32, tag="rr", name="rr")
                        nc.vector.reciprocal_approx_fast(out=rr[:],
                                                         in_=o_ps[0:1, :, :])
                        rrr = nrm.tile([1, CPB, P], F32R, tag="rrr",
                                       name="rrr")
                        nc.vector.tensor_copy(out=rrr[:], in_=rr[:])
                        rrrs.append(rrr)
                        oc = nrm.tile([HD + 1, CPB, P], F32, tag="oc",
                                      name="oc")
                        nc.vector.tensor_copy(out=oc[:], in_=o_ps[:, :, :])
                        ocs.append(oc)
                    rb = m_ps.tile([P, CPB, P], F32, tag="m", name="rb")
                    for h in range(2):
                        nc.tensor.matmul(rb[0:HD + 1, :, :],
                                         lhsT=ONES[0:1, :], rhs=rrrs[h][:],
                                         start=True, stop=True)
                        on = nrm.tile([HD + 1, CPB, P], BF16, tag="on",
                                      name="on")
                        nc.vector.tensor_mul(out=on[:],
                                             in0=ocs[h][:, :, :],
                                             in1=rb[0:HD + 1, :, :])
                        dst = a2a_in[c][b, :, h * HD:(h + 1) * HD, :]
                        nc.sync.dma_start(dst.transpose([1, 0, 2]),
                                          on[1:HD + 1, :, :])
                pending.append(finisher)

            # ---- main schedule ----
            def a2a_round(r):
                def trig():
                    if no_cc:
                        return
                    nc.gpsimd.collective_compute(
                        "AllToAll", mybir.AluOpType.bypass,
                        replica_groups=[list(range(NCORES))],
                        ins=[a2a_in[r][:, :, :, :].opt()],
                        outs=[a2a_out[r][:, :, :].opt()],
                    )
                pending.append(trig)

            for c in range(CPB):
                for b in range(B):
                    ensure(list(range(4 * b, 4 * b + c + 1)))
                    emit_attn(c, b)
                a2a_round(c)
            flush_pending()
            while _advance():
                pass
            for r in range(CPB - 1):
                emit_oproj(r)
            # keep the PE streaming (HAM warm) while the tail AllToAll is in
            # flight; results are never read.
            for g in range(3):
                dm = m_ps.tile([P, 512], F32, tag="m", name="dm")
                for kt in range(NKT):
                    nc.tensor.matmul(dm[:], lhsT=WO[:, kt, 0:P],
                                     rhs=XT[:, 0, kt, :],
                                     start=(kt == 0), stop=(kt == NKT - 1))
            emit_oproj(CPB - 1)

        for f in frees:
            f()
        res.release()
    nc.finalize()
    return nc


@functools.cache
def _get_nc():
    return build_kernel()


def _bf(a):
    return np.asarray(a, np.float32).astype(ml_dtypes.bfloat16)


def _prep_w(w_shard):
    # [128 out-feat, 1024 in] -> lhsT tiles [p, kt, m]: w[p,kt,m]=W[m, kt*128+p]
    return np.ascontiguousarray(
        _bf(w_shard).T.reshape(NKT, P, w_shard.shape[0]).transpose(1, 0, 2))


_last_in_maps = None


def kernel(x, mask, Wq, Wk, Wv, Wo):
    x = np.asarray(x, np.float32)

    xt = _bf(x).reshape(TOK, DIM).T            # [D, TOK] bf16
    xt = np.ascontiguousarray(xt).reshape(NKT, P, NTC, 512)
    xT = np.ascontiguousarray(xt.transpose(2, 1, 0, 3))  # [tch, p, kt, 512]
    wo_t = np.ascontiguousarray(
        _bf(Wo).T.reshape(NKT, P, DIM).transpose(1, 0, 2))
    tri = np.triu(np.ones((P, P), np.float32)).astype(ml_dtypes.bfloat16)
    idn = np.eye(P, dtype=np.float32).astype(ml_dtypes.bfloat16)

    in_maps = []
    for c in range(NCORES):
        sl = slice(c * P, (c + 1) * P)
        in_maps.append(dict(
            xT=xT,
            wq=_prep_w(np.asarray(Wq, np.float32)[sl]),
            wk=_prep_w(np.asarray(Wk, np.float32)[sl]),
            wv=_prep_w(np.asarray(Wv, np.float32)[sl]),
            wo=wo_t,
            tri=tri,
            idn=idn,
        ))

    nc = _get_nc()
    global _last_in_maps
    _last_in_maps = in_maps
    res = bass_utils.run_bass_kernel_spmd(nc, in_maps,
                                          core_ids=list(range(NCORES)))
    full = np.empty((TOK, DIM), np.float32)
    for j in range(NCORES):
        o = res.results[j]["out"]          # [512, 1024], rows = 4 rounds x 128
        bb = j // 4
        for c in range(CPB):
            t0 = 512 * c + P * (j % 4)
            full[bb * T + t0: bb * T + t0 + P] = o[c * P:(c + 1) * P]
    return full.reshape(B, T, DIM)


if __name__ == "__main__":
    rng = np.random.default_rng(0)
    x = rng.standard_normal((B, T, DIM)).astype(np.float32)
    neg = np.finfo(np.float32).min
    mask = np.triu(np.full((T, T), neg, np.float32), k=1)[None, None]
    Ws = [(rng.standard_normal((DIM, DIM)) * 0.02).astype(np.float32)
          for _ in range(4)]
    out = kernel(x, mask, *Ws)
    print("out", out.shape, out.dtype, np.abs(out).max())
